# revision 1
# baseline (speedup 1.0000x reference)
"""Distributed single-head attention on 8 TRN2 NeuronCores.

Math (matches the reference):
    q = z @ Wq; k = z @ Wk; v = z @ Wv
    out = softmax(q k^T) * DK**-0.5 @ v

Sharding: z rows split 8 ways. Each core projects its own shard; the
K^T (fp16) and V (bf16) shards are all-gathered via three async
AllGathers (K^T halves early, V later) that each complete before any
core needs the data, hiding the collective behind local projection
work. Then each core does flash-style row-block attention:
    S^T_j = K^T[:, j-tile] ^T-matmul Q^T           (fp16 operands, f32 PSUM)
    P_j   = exp(S^T_j - 40)                        (bf16, shift-invariant)
    rowsumT = ones^T @ P                           (PE)
    out   = (P^T-matmuls V) * (scale / rowsum)
Layouts keep seq on partitions for P so both matmuls are native
(no transposes anywhere; z^T is prepared on the host).

Precision: fp16 z/W/Q/K + f32 PSUM keeps logits to ~1e-2 abs err;
exp/V/AV in bf16. End-to-end rel err ~3e-3 (vs f32 reference).
"""

import numpy as np

SEQ, D, DK, DV = 4096, 1024, 1024, 1024
NCORES = 8
ROWS = SEQ // NCORES            # 512 rows per core
DT = D // 128                   # 8 contraction tiles (input dim)
MT = DK // 128                  # 8 dk tiles
ST = ROWS // 128                # 4 local seq tiles
JT = SEQ // 128                 # 32 global seq tiles
SHIFT = 40.0                    # constant logit shift (softmax-invariant)
SCALE = DK ** -0.5

KT_ELEMS = DK * ROWS            # fp16 K^T shard elems in packed bounce
PACK_ELEMS = KT_ELEMS + ROWS * DV


def _build():
    import concourse.mybir as mybir
    import concourse.tile as tile
    from concourse import bacc

    F32 = mybir.dt.float32
    F16 = mybir.dt.float16
    BF16 = mybir.dt.bfloat16
    Exp = mybir.ActivationFunctionType.Exp

    nc = bacc.Bacc("TRN2", target_bir_lowering=False, debug=False, num_devices=NCORES)
    d_zT = nc.declare_dram_parameter("zT", [D, ROWS], F16, isOutput=False)
    d_wq = nc.declare_dram_parameter("Wq", [D, DK], F16, isOutput=False)
    d_wk = nc.declare_dram_parameter("Wk", [D, DK], F16, isOutput=False)
    d_wv = nc.declare_dram_parameter("Wv", [D, DV], F16, isOutput=False)
    d_out = nc.declare_dram_parameter("out", [ROWS, DV], F32, isOutput=True)

    with tile.TileContext(nc) as tc:
        with (
            tc.tile_pool(name="dram", bufs=1, space="DRAM") as dram,
            tc.tile_pool(name="qt", bufs=1) as qt_pool,
            tc.tile_pool(name="misc", bufs=1) as misc,
            tc.tile_pool(name="stage", bufs=4) as stage,
            tc.tile_pool(name="ps_proj", bufs=2, space="PSUM") as ps_proj,
            tc.tile_pool(name="ps_s", bufs=2, space="PSUM") as ps_s,
            tc.tile_pool(name="ps_rs", bufs=1, space="PSUM") as ps_rs,
            tc.tile_pool(name="ps_o", bufs=2, space="PSUM") as ps_o,
            tc.tile_pool(name="outp", bufs=2) as outp,
        ):
            # ---- bounce + collective buffers.  Three async gathers, each
            # triggered as soon as its producer finishes, so every one
            # completes before any core needs its data (absorbs the ~30us
            # cross-core launch skew).
            KT_H = KT_ELEMS // 2
            kt1_in = dram.tile([KT_H], BF16)
            kt1_out = dram.tile([NCORES * KT_H], BF16, addr_space="Shared")
            kt2_in = dram.tile([KT_H], BF16)
            kt2_out = dram.tile([NCORES * KT_H], BF16, addr_space="Shared")
            v_in = dram.tile([ROWS * DV], BF16)
            v_out = dram.tile([NCORES * ROWS * DV], BF16, addr_space="Shared")

            ones_sb = misc.tile([128, 1], BF16)
            nc.vector.memset(ones_sb[:], 1.0)
            # full-width ones: M=128 rowsum matmuls run at the standard
            # N=512 rate (M=1 measured ~40% slower); every psum row gets
            # the same rowsum vector, row 0 is read out
            ones128 = misc.tile([128, 128], BF16)
            nc.vector.memset(ones128[:], 1.0)
            bias_sb = misc.tile([128, 1], F32)
            nc.vector.memset(bias_sb[:], -SHIFT)
            # touch Exp once so the ACT table set loads during proj
            warm_sb = misc.tile([128, 1], F32)
            nc.scalar.activation(warm_sb[:], ones_sb[:], Exp,
                                 bias=bias_sb[:], scale=1.0)

            # HAM pre-warm: keep the PE busy through the ~12us weight-DMA
            # head (idle >3.4us re-throttles the PE clock to 1.2GHz, which
            # would make the first proj matmuls run at half rate)
            with (
                tc.tile_pool(name="warmmm", bufs=1) as warm_pool,
                tc.tile_pool(name="ps_warm", bufs=1, space="PSUM") as ps_warm,
            ):
                wsrc = warm_pool.tile([128, 512], BF16)
                nc.vector.memset(wsrc[:], 0.0)
                wps = ps_warm.tile([128, 512], F32)
                for _ in range(32):
                    nc.tensor.matmul(wps[:], wsrc[:, 0:128], wsrc[:],
                                     start=True, stop=True)

            # ---------------- projection phase (scoped weights) ----------
            with tc.tile_pool(name="wz", bufs=1) as wz:
                # per-chunk tiles so each proj matmul depends only on the
                # one DMA that feeds it (whole-tile deps stalled proj)
                zv = d_zT.rearrange("(t p) n -> p t n", p=128)
                zT_sb = []
                for t in range(DT):
                    zt = wz.tile([128, ROWS], F16, name=f"zt{t}")
                    nc.sync.dma_start(zt[:], zv[:, t, :])
                    zT_sb.append(zt)

                def load_w(d_w, prefix):
                    tiles = []
                    wvw = d_w.rearrange("(t p) m -> p t m", p=128)
                    for t in range(DT):
                        w = wz.tile([128, DK], F16, name=f"{prefix}{t}")
                        nc.sync.dma_start(w[:], wvw[:, t, :])
                        tiles.append(w)
                    return tiles

                wk_sb = load_w(d_wk, "wk")
                wv_sb = load_w(d_wv, "wv")
                wq_sb = load_w(d_wq, "wq")

                # K^T shard: [DK, ROWS] fp16, split by SEQ columns so the
                # first gather alone supports half the S-phase j-tiles
                # (a dk split would leave every S chain waiting on both).
                HN = ROWS // 2
                ktv1 = kt1_in[:].rearrange("(m p n) -> p m n", p=128, n=HN)
                ktv2 = kt2_in[:].rearrange("(m p n) -> p m n", p=128, n=HN)
                for m in range(MT):
                    pk = ps_proj.tile([128, 512], F32, tag="psproj")
                    for t in range(DT):
                        nc.tensor.matmul(pk[:], wk_sb[t][:, m * 128:(m + 1) * 128],
                                         zT_sb[t][:],
                                         start=(t == 0), stop=(t == DT - 1))
                    kt_stage = stage.tile([128, ROWS], F16, tag="ktstage")
                    nc.vector.tensor_copy(kt_stage[:], pk[:])
                    nc.sync.dma_start(ktv1[:, m, :],
                                      kt_stage[:, 0:HN].bitcast(BF16))
                    nc.sync.dma_start(ktv2[:, m, :],
                                      kt_stage[:, HN:ROWS].bitcast(BF16))
                nc.gpsimd.collective_compute(
                    "AllGather", mybir.AluOpType.bypass,
                    replica_groups=[list(range(NCORES))],
                    ins=[kt1_in[:].opt()], outs=[kt1_out[:].opt()])
                nc.gpsimd.collective_compute(
                    "AllGather", mybir.AluOpType.bypass,
                    replica_groups=[list(range(NCORES))],
                    ins=[kt2_in[:].opt()], outs=[kt2_out[:].opt()])

                # V shard: [ROWS, DV] bf16 -> v_in
                vv = v_in[:].rearrange("(s p m) -> p s m", p=128, m=DV)
                for s in range(ST):
                    for h in range(2):
                        pv = ps_proj.tile([128, 512], F32, tag="psproj")
                        for t in range(DT):
                            nc.tensor.matmul(
                                pv[:], zT_sb[t][:, s * 128:(s + 1) * 128],
                                wv_sb[t][:, h * 512:(h + 1) * 512],
                                start=(t == 0), stop=(t == DT - 1))
                        v_stage = stage.tile([128, 512], BF16, tag="vstage")
                        nc.vector.tensor_copy(v_stage[:], pv[:])
                        nc.sync.dma_start(vv[:, s, h * 512:(h + 1) * 512], v_stage[:])

                nc.gpsimd.collective_compute(
                    "AllGather", mybir.AluOpType.bypass,
                    replica_groups=[list(range(NCORES))],
                    ins=[v_in[:].opt()], outs=[v_out[:].opt()])

                # Q^T: [DK, ROWS] fp16, resident (overlaps the collective)
                qt_sb = qt_pool.tile([128, MT, ROWS], F16)
                for m in range(MT):
                    pq = ps_proj.tile([128, 512], F32, tag="psproj")
                    for t in range(DT):
                        nc.tensor.matmul(pq[:], wq_sb[t][:, m * 128:(m + 1) * 128],
                                         zT_sb[t][:],
                                         start=(t == 0), stop=(t == DT - 1))
                    nc.vector.tensor_copy(qt_sb[:, m, :], pq[:])

            # ---------------- gathered tiles ------------------------------
            with (
                tc.tile_pool(name="ktg", bufs=4) as ktg_pool,
                tc.tile_pool(name="vg", bufs=1) as vg_pool,
                tc.tile_pool(name="expp", bufs=1) as expp,
            ):
                # V gathered: resident [128, JT, DV] bf16 (64KB/partition)
                v_sb = vg_pool.tile([128, JT, DV], BF16)
                expS = expp.tile([128, JT, ROWS], BF16)
                rs_ps = ps_rs.tile([128, 512], F32)

                # Two passes: pass 0 runs on the first-gathered seq half
                # (j-tiles 0,1 of every block) while the second gather and
                # V land; pass 1 finishes j-tiles 2,3.
                HN = ROWS // 2
                n_rs = 0
                for half, kt_out_h in ((0, kt1_out), (1, kt2_out)):
                    for b in range(NCORES):
                        ktb = ktg_pool.tile([128, MT, HN], F16, tag="ktg")
                        src = kt_out_h[b * KT_H:(b + 1) * KT_H].rearrange(
                            "(m p n) -> p m n", p=128, n=HN).bitcast(F16)
                        nc.sync.dma_start(ktb[:, 0:4, :], src[:, 0:4, :])
                        nc.sync.dma_start(ktb[:, 4:8, :], src[:, 4:8, :])
                        if half == 0:
                            # V loads ride the gpsimd (SWDGE) queues so they
                            # never head-of-line block the K^T loads.
                            vsrc = v_out[b * ROWS * DV:
                                         (b + 1) * ROWS * DV].rearrange(
                                "(s p m) -> p s m", p=128, m=DV)
                            nc.gpsimd.dma_start(
                                v_sb[:, b * ST:(b + 1) * ST, :], vsrc)

                        for jj in range(2):
                            j = b * ST + half * 2 + jj
                            ps_S = ps_s.tile([128, 512], F32, tag="pss")
                            for t in range(MT):
                                nc.tensor.matmul(
                                    ps_S[:],
                                    ktb[:, t, jj * 128:(jj + 1) * 128],
                                    qt_sb[:, t, :],
                                    start=(t == 0), stop=(t == MT - 1))
                            nc.scalar.activation(expS[:, j, :], ps_S[:], Exp,
                                                 bias=bias_sb[:], scale=1.0)
                            nc.tensor.matmul(rs_ps[:], ones128[:],
                                             expS[:, j, :],
                                             start=(n_rs == 0),
                                             stop=(n_rs == JT - 1))
                            n_rs += 1

                # row-sum -> per-row reciprocal multipliers [128, ST]
                rs_sb = misc.tile([1, 512], F32)
                nc.vector.tensor_copy(rs_sb[:], rs_ps[0:1, :])
                rs_dram = dram.tile([1, 512], F32)
                nc.sync.dma_start(rs_dram[:], rs_sb[:])
                rs128 = misc.tile([128, ST], F32)
                nc.sync.dma_start(
                    rs128[:], rs_dram[0, :].rearrange("(r p) -> p r", p=128))
                mult_sb = misc.tile([128, ST], F32)
                nc.vector.reciprocal(mult_sb[:], rs128[:])
                nc.vector.tensor_scalar_mul(mult_sb[:], mult_sb[:], SCALE)

                # ---------------- AV phase ---------------------------------
                for h in range(2):
                    for r in range(ST):
                        po = ps_o.tile([128, 512], F32, tag="pso")
                        for j in range(JT):
                            nc.tensor.matmul(
                                po[:],
                                expS[:, j, r * 128:(r + 1) * 128],
                                v_sb[:, j, h * 512:(h + 1) * 512],
                                start=(j == 0), stop=(j == JT - 1))
                        o_sb = outp.tile([128, 512], F32, tag="osb")
                        nc.vector.tensor_scalar_mul(o_sb[:], po[:],
                                                    mult_sb[:, r:r + 1])
                        nc.sync.dma_start(
                            d_out[r * 128:(r + 1) * 128, h * 512:(h + 1) * 512],
                            o_sb[:])
    nc.compile()
    return nc


_BUILT = None


def kernel(z, Wq, Wk, Wv):
    global _BUILT
    from concourse.bass_utils import run_bass_kernel_spmd

    if _BUILT is None:
        _BUILT = _build()
    nc = _BUILT

    zT = np.ascontiguousarray(z.T).astype(np.float16)
    wq16 = Wq.astype(np.float16)
    wk16 = Wk.astype(np.float16)
    wv16 = Wv.astype(np.float16)
    in_maps = [
        {
            "zT": np.ascontiguousarray(zT[:, c * ROWS:(c + 1) * ROWS]),
            "Wq": wq16,
            "Wk": wk16,
            "Wv": wv16,
        }
        for c in range(NCORES)
    ]
    res = run_bass_kernel_spmd(nc, in_maps, list(range(NCORES)))
    out = np.concatenate([res.results[c]["out"] for c in range(NCORES)], axis=0)
    return out.astype(np.float32)


if __name__ == "__main__":
    rng = np.random.default_rng(0)
    z = rng.standard_normal((SEQ, D)).astype(np.float32)
    Wq = (0.02 * rng.standard_normal((D, DK))).astype(np.float32)
    Wk = (0.02 * rng.standard_normal((D, DK))).astype(np.float32)
    Wv = (0.02 * rng.standard_normal((D, DV))).astype(np.float32)
    out = kernel(z=z, Wq=Wq, Wk=Wk, Wv=Wv)
    print(out.shape, out.dtype)



# revision 2
# speedup vs baseline: 1.0067x; 1.0067x over previous
"""Distributed single-head attention on 8 TRN2 NeuronCores.

Math (matches the reference):
    q = z @ Wq; k = z @ Wk; v = z @ Wv
    out = softmax(q k^T) * DK**-0.5 @ v

Sharding: z rows split 8 ways. Each core projects its own shard; K^T
(fp16) and V (bf16) shards are all-gathered in four async halves
(kt1, kt2, vA, vB — split along local seq) so each gather triggers as
early as possible and lands before its consumer phase. Flash-style
row-block attention follows:
    S^T_j = K^T[:, j-tile] ^T-matmul Q^T           (fp16 operands, f32 PSUM)
    P_j   = exp(S^T_j - 40)                        (bf16, shift-invariant)
    rowsumT = ones^T @ P                           (PE)
    out   = (P^T-matmuls V) * (scale / rowsum)

Schedule: PE starts real work ~7.5us in (t-outer K projection over all
8 PSUM banks needs only the first zT/Wk chunk pair); Wv/Wq input loads
ride SWDGE so the K^T stage writes aren't queued behind them on the
HWDGE rings; the AV phase accumulates in two passes (j%4 in {0,1},
then {2,3}) so the second V gather may land ~30us into AV.

Precision: fp16 z/W/Q/K + f32 PSUM keeps logits to ~1e-2 abs err;
exp/V/AV in bf16. End-to-end rel err ~3e-3 (vs f32 reference).
"""

import numpy as np

SEQ, D, DK, DV = 4096, 1024, 1024, 1024
NCORES = 8
ROWS = SEQ // NCORES            # 512 rows per core
DT = D // 128                   # 8 contraction tiles (input dim)
MT = DK // 128                  # 8 dk tiles
ST = ROWS // 128                # 4 local seq tiles
JT = SEQ // 128                 # 32 global seq tiles
HN = ROWS // 2                  # 256 = half the local rows
SHIFT = 40.0                    # constant logit shift (softmax-invariant)
SCALE = DK ** -0.5

KT_H = DK * HN                  # elems in one K^T half (bf16-bitcast fp16)
V_H = HN * DV                   # elems in one V half (bf16)


def _build():
    import concourse.mybir as mybir
    import concourse.tile as tile
    from concourse import bacc

    F32 = mybir.dt.float32
    F16 = mybir.dt.float16
    BF16 = mybir.dt.bfloat16
    Exp = mybir.ActivationFunctionType.Exp

    nc = bacc.Bacc("TRN2", target_bir_lowering=False, debug=False, num_devices=NCORES)
    d_zT = nc.declare_dram_parameter("zT", [D, ROWS], F16, isOutput=False)
    d_wq = nc.declare_dram_parameter("Wq", [D, DK], F16, isOutput=False)
    d_wk = nc.declare_dram_parameter("Wk", [D, DK], F16, isOutput=False)
    d_wv = nc.declare_dram_parameter("Wv", [D, DV], F16, isOutput=False)
    d_out = nc.declare_dram_parameter("out", [ROWS, DV], F32, isOutput=True)

    rg = [list(range(NCORES))]

    with tile.TileContext(nc) as tc:
        with (
            tc.tile_pool(name="dram", bufs=1, space="DRAM") as dram,
            tc.tile_pool(name="qt", bufs=1) as qt_pool,
            tc.tile_pool(name="misc", bufs=1) as misc,
            tc.tile_pool(name="stage", bufs=4) as stage,
            tc.tile_pool(name="vg", bufs=1) as vg_pool,
            tc.tile_pool(name="expp", bufs=1) as expp,
            tc.tile_pool(name="outp", bufs=2) as outp,
        ):
            # ---- collective bounce buffers (per local-seq half) ----
            kt1_in = dram.tile([KT_H], BF16)
            kt1_out = dram.tile([NCORES * KT_H], BF16, addr_space="Shared")
            kt2_in = dram.tile([KT_H], BF16)
            kt2_out = dram.tile([NCORES * KT_H], BF16, addr_space="Shared")
            va_in = dram.tile([V_H], BF16)
            va_out = dram.tile([NCORES * V_H], BF16, addr_space="Shared")
            vb_in = dram.tile([V_H], BF16)
            vb_out = dram.tile([NCORES * V_H], BF16, addr_space="Shared")

            # constants: full-width ones for the PE rowsum (M=128 runs at
            # standard N=512 rate; M=1 measured ~40% slower), exp bias
            ones128 = misc.tile([128, 128], BF16)
            nc.vector.memset(ones128[:], 1.0)
            bias_sb = misc.tile([128, 1], F32)
            nc.vector.memset(bias_sb[:], -SHIFT)
            # touch Exp once so the ACT table set loads before the S phase
            warm_sb = misc.tile([128, 1], F32)
            nc.scalar.activation(warm_sb[:], bias_sb[:], Exp,
                                 bias=bias_sb[:], scale=1.0)

            # ---------------- projection phase (scoped weights) ----------
            with (
                tc.tile_pool(name="wz", bufs=1) as wz,
                tc.tile_pool(name="ps_proj", bufs=8, space="PSUM") as ps_proj,
            ):
                # zT + Wk chunk pairs on the HWDGE (sync) rings, interleaved
                # so the t-outer K projection can start after one pair lands
                zv = d_zT.rearrange("(t p) n -> p t n", p=128)
                wkv = d_wk.rearrange("(t p) m -> p t m", p=128)
                zT_sb, wk_sb = [], []
                for t in range(DT):
                    zt = wz.tile([128, ROWS], F16, name=f"zt{t}")
                    nc.sync.dma_start(zt[:], zv[:, t, :])
                    zT_sb.append(zt)
                    w = wz.tile([128, DK], F16, name=f"wk{t}")
                    nc.sync.dma_start(w[:], wkv[:, t, :])
                    wk_sb.append(w)
                # Wv/Wq ride SWDGE queues: separate rings, so the K^T/V
                # stage writes below aren't queued behind 4MB of weights
                wv_sb, wq_sb = [], []
                for d_w, prefix, tiles in ((d_wv, "wv", wv_sb), (d_wq, "wq", wq_sb)):
                    wvw = d_w.rearrange("(t p) m -> p t m", p=128)
                    for t in range(DT):
                        w = wz.tile([128, DK], F16, name=f"{prefix}{t}")
                        nc.gpsimd.dma_start(w[:], wvw[:, t, :])
                        tiles.append(w)

                # K^T shard [DK, ROWS], two seq-halves; t-outer over all 8
                # PSUM banks so the first matmul needs only (zt0, wk0)
                for half, kt_in in ((0, kt1_in), (1, kt2_in)):
                    psk = [ps_proj.tile([128, HN], F32, tag="psproj",
                                        name=f"psk{half}{m}") for m in range(MT)]
                    for t in range(DT):
                        for m in range(MT):
                            nc.tensor.matmul(
                                psk[m][:], wk_sb[t][:, m * 128:(m + 1) * 128],
                                zT_sb[t][:, half * HN:(half + 1) * HN],
                                start=(t == 0), stop=(t == DT - 1))
                    ktv = kt_in[:].rearrange("(m p n) -> p m n", p=128, n=HN)
                    for m in range(MT):
                        kt_stage = stage.tile([128, HN], F16, tag="ktstage")
                        nc.vector.tensor_copy(kt_stage[:], psk[m][:])
                        nc.sync.dma_start(ktv[:, m, :], kt_stage[:].bitcast(BF16))
                    nc.gpsimd.collective_compute(
                        "AllGather", mybir.AluOpType.bypass, replica_groups=rg,
                        ins=[kt_in[:].opt()],
                        outs=[(kt1_out if half == 0 else kt2_out)[:].opt()])

                # V shard [ROWS, DV] bf16, (s,h)-outer so stage writes
                # stagger; gather each seq-half as soon as it's staged
                vva = va_in[:].rearrange("(s p m) -> p s m", p=128, m=DV)
                vvb = vb_in[:].rearrange("(s p m) -> p s m", p=128, m=DV)
                for s in range(ST):
                    for h in range(2):
                        pv = ps_proj.tile([128, 512], F32, tag="psproj")
                        for t in range(DT):
                            nc.tensor.matmul(
                                pv[:], zT_sb[t][:, s * 128:(s + 1) * 128],
                                wv_sb[t][:, h * 512:(h + 1) * 512],
                                start=(t == 0), stop=(t == DT - 1))
                        v_stage = stage.tile([128, 512], BF16, tag="vstage")
                        nc.vector.tensor_copy(v_stage[:], pv[:])
                        vv = vva if s < 2 else vvb
                        nc.sync.dma_start(vv[:, s % 2, h * 512:(h + 1) * 512],
                                          v_stage[:])
                    if s == 1:
                        nc.gpsimd.collective_compute(
                            "AllGather", mybir.AluOpType.bypass, replica_groups=rg,
                            ins=[va_in[:].opt()], outs=[va_out[:].opt()])
                nc.gpsimd.collective_compute(
                    "AllGather", mybir.AluOpType.bypass, replica_groups=rg,
                    ins=[vb_in[:].opt()], outs=[vb_out[:].opt()])

                # Q^T: [DK, ROWS] fp16, resident (overlaps the collectives)
                qt_sb = qt_pool.tile([128, MT, ROWS], F16)
                for m in range(MT):
                    pq = ps_proj.tile([128, 512], F32, tag="psproj")
                    for t in range(DT):
                        nc.tensor.matmul(pq[:], wq_sb[t][:, m * 128:(m + 1) * 128],
                                         zT_sb[t][:],
                                         start=(t == 0), stop=(t == DT - 1))
                    nc.vector.tensor_copy(qt_sb[:, m, :], pq[:])

            # V gathered: resident [128, JT, DV] bf16 (64KB/partition).
            # Loads ride SWDGE so they never head-of-line block the
            # HWDGE K^T loads; issued in (vA, vB) order = consumption order.
            v_sb = vg_pool.tile([128, JT, DV], BF16)
            for b in range(NCORES):
                src = va_out[b * V_H:(b + 1) * V_H].rearrange(
                    "(s p m) -> p s m", p=128, m=DV)
                nc.gpsimd.dma_start(v_sb[:, b * ST:b * ST + 2, :], src)
            for b in range(NCORES):
                src = vb_out[b * V_H:(b + 1) * V_H].rearrange(
                    "(s p m) -> p s m", p=128, m=DV)
                nc.gpsimd.dma_start(v_sb[:, b * ST + 2:b * ST + 4, :], src)

            expS = expp.tile([128, JT, ROWS], BF16)

            # ---------------- S phase -------------------------------------
            with (
                tc.tile_pool(name="ktg", bufs=4) as ktg_pool,
                tc.tile_pool(name="ps_s", bufs=4, space="PSUM") as ps_s,
                tc.tile_pool(name="ps_rs", bufs=1, space="PSUM") as ps_rs,
            ):
                rs_ps = ps_rs.tile([128, 512], F32)
                n_rs = 0
                for half, kt_out_h in ((0, kt1_out), (1, kt2_out)):
                    for b in range(NCORES):
                        ktb = ktg_pool.tile([128, MT, HN], F16, tag="ktg")
                        src = kt_out_h[b * KT_H:(b + 1) * KT_H].rearrange(
                            "(m p n) -> p m n", p=128, n=HN).bitcast(F16)
                        nc.sync.dma_start(ktb[:, 0:4, :], src[:, 0:4, :])
                        nc.sync.dma_start(ktb[:, 4:8, :], src[:, 4:8, :])
                        for jj in range(2):
                            j = b * ST + half * 2 + jj
                            ps_S = ps_s.tile([128, 512], F32, tag="pss")
                            for t in range(MT):
                                nc.tensor.matmul(
                                    ps_S[:],
                                    ktb[:, t, jj * 128:(jj + 1) * 128],
                                    qt_sb[:, t, :],
                                    start=(t == 0), stop=(t == MT - 1))
                            nc.scalar.activation(expS[:, j, :], ps_S[:], Exp,
                                                 bias=bias_sb[:], scale=1.0)
                            nc.tensor.matmul(rs_ps[:], ones128[:],
                                             expS[:, j, :],
                                             start=(n_rs == 0),
                                             stop=(n_rs == JT - 1))
                            n_rs += 1

                # row-sum -> per-row reciprocal multipliers [128, ST]
                rs_sb = misc.tile([1, 512], F32)
                nc.vector.tensor_copy(rs_sb[:], rs_ps[0:1, :])
                rs_dram = dram.tile([1, 512], F32)
                nc.sync.dma_start(rs_dram[:], rs_sb[:])
                rs128 = misc.tile([128, ST], F32)
                nc.sync.dma_start(
                    rs128[:], rs_dram[0, :].rearrange("(r p) -> p r", p=128))
                mult_sb = misc.tile([128, ST], F32)
                nc.vector.reciprocal(mult_sb[:], rs128[:])
                nc.vector.tensor_scalar_mul(mult_sb[:], mult_sb[:], SCALE)

            # ---------------- AV phase ------------------------------------
            # Two accumulation passes over all 8 (h, r) PSUM banks:
            # pass 0 consumes j%4 in {0,1} (vA), pass 1 j%4 in {2,3} (vB) —
            # so vB may land up to ~30us into the AV phase without stalling.
            j_pass = ([j for j in range(JT) if j % 4 < 2],
                      [j for j in range(JT) if j % 4 >= 2])
            with tc.tile_pool(name="ps_o", bufs=8, space="PSUM") as ps_o:
                po = [ps_o.tile([128, 512], F32, tag="pso", name=f"po{g}")
                      for g in range(8)]
                for part in (0, 1):
                    for h in range(2):
                        for r in range(ST):
                            p = po[h * ST + r]
                            for idx, j in enumerate(j_pass[part]):
                                nc.tensor.matmul(
                                    p[:],
                                    expS[:, j, r * 128:(r + 1) * 128],
                                    v_sb[:, j, h * 512:(h + 1) * 512],
                                    start=(part == 0 and idx == 0),
                                    stop=(part == 1 and idx == len(j_pass[1]) - 1))
                            if part == 1:
                                o_sb = outp.tile([128, 512], F32, tag="osb")
                                nc.vector.tensor_scalar_mul(o_sb[:], p[:],
                                                            mult_sb[:, r:r + 1])
                                nc.sync.dma_start(
                                    d_out[r * 128:(r + 1) * 128,
                                          h * 512:(h + 1) * 512],
                                    o_sb[:])
    nc.compile()
    return nc


_BUILT = None


def kernel(z, Wq, Wk, Wv):
    global _BUILT
    from concourse.bass_utils import run_bass_kernel_spmd

    if _BUILT is None:
        _BUILT = _build()
    nc = _BUILT

    zT = np.ascontiguousarray(z.T).astype(np.float16)
    wq16 = Wq.astype(np.float16)
    wk16 = Wk.astype(np.float16)
    wv16 = Wv.astype(np.float16)
    in_maps = [
        {
            "zT": np.ascontiguousarray(zT[:, c * ROWS:(c + 1) * ROWS]),
            "Wq": wq16,
            "Wk": wk16,
            "Wv": wv16,
        }
        for c in range(NCORES)
    ]
    res = run_bass_kernel_spmd(nc, in_maps, list(range(NCORES)))
    out = np.concatenate([res.results[c]["out"] for c in range(NCORES)], axis=0)
    return out.astype(np.float32)


if __name__ == "__main__":
    rng = np.random.default_rng(0)
    z = rng.standard_normal((SEQ, D)).astype(np.float32)
    Wq = (0.02 * rng.standard_normal((D, DK))).astype(np.float32)
    Wk = (0.02 * rng.standard_normal((D, DK))).astype(np.float32)
    Wv = (0.02 * rng.standard_normal((D, DV))).astype(np.float32)
    out = kernel(z=z, Wq=Wq, Wk=Wk, Wv=Wv)
    print(out.shape, out.dtype)
